# revision 1
# baseline (speedup 1.0000x reference)
"""Trainium2 Bass kernel for ContinuousDGM message passing.

  xe = x @ W_emb + b_emb            [N, E]
  D  = sq_cdist(xe)                 [N, N]
  A  = 1 / (1 + D)
  W  = A / A.sum(axis=1)            (broadcast over last axis -> col-normalize)
  out = W @ xe                      [N, E]

Strategy (8 NeuronCores, row-block sharding, fully fused -- the [N,N]
matrices never touch DRAM):
  * Host passes x already transposed (xT [DIN, N]) plus the core's own
    column block (xTl), so every matmul has its contraction dim on
    partitions with no on-device transpose of x.
  * On device, augmented operand buffers augL/augR [68, N] bf16 hold
    [-2*xeT; ones; ones; sq_hi; sq_lo] so ONE matmul produces
    psum = 1 + sq_i + sq_j - 2*G = 1 + D for any tile.  sq rides as a
    bf16 hi+lo pair (error ~3e-4) and is computed from the *same* bf16
    xe values the PE multiplies, so the diagonal cancels to ~1e-3
    without any masking.
  * A = reciprocal(psum) on DVE (approx_fast, ~18 bits).
  * Pass 1: local row sums s (symmetry => col sums), tiny AllGather of
    1/s (4KB/core).  Pass 2: recompute A^T tiles, matmul with
    ye = xe * (1/s) accumulating out^T [E, B] per core in PSUM.
  * Host concatenates the 8 out^T blocks and transposes.
"""

import os
import sys

import numpy as np

N, DIN, E = 8192, 256, 64
P = 128
C = 8
B = N // C            # 1024 rows per core
SUP = 512
NSUP = N // SUP       # 16
BSUP = B // SUP       # 2
NT = N // P           # 64
BT = B // P           # 8

_NC_CACHE = {}


def _import_concourse():
    try:
        import concourse.bacc  # noqa: F401
    except ImportError:
        for p in ("/opt/trn_rl_repo", "/root/.axon_site/_ro/trn_rl_repo"):
            if os.path.isdir(p) and p not in sys.path:
                sys.path.insert(0, p)
        import concourse.bacc  # noqa: F401


def build_body(tc, outT, xT, xTl, W, b, eye):
    """Emit the kernel body. All args are bass APs of DRAM tensors."""
    from contextlib import ExitStack

    import concourse.bass as bass  # noqa: F401
    from concourse import mybir

    nc = tc.nc
    f32 = mybir.dt.float32
    bf16 = mybir.dt.bfloat16
    AF = mybir.ActivationFunctionType
    ALU = mybir.AluOpType
    AX = mybir.AxisListType

    with ExitStack() as ctx:
        big = ctx.enter_context(tc.tile_pool(name="big", bufs=1))
        const = ctx.enter_context(tc.tile_pool(name="const", bufs=1))
        work = ctx.enter_context(tc.tile_pool(name="work", bufs=1))
        psum = ctx.enter_context(tc.tile_pool(name="psum", bufs=1, space="PSUM"))
        dram = ctx.enter_context(tc.tile_pool(name="dram", bufs=1, space="DRAM"))

        # ---------- load inputs ----------
        Wsb = const.tile([P, 2, E], f32, name="Wsb", tag="Wsb")
        for t in range(2):
            nc.sync.dma_start(Wsb[:, t, :], W[t * P:(t + 1) * P, :])
        b_col = const.tile([E, 1], f32, name="bcol", tag="bcol")
        nc.sync.dma_start(b_col[:], b[:])
        b2_col = const.tile([E, 1], f32, name="b2col", tag="b2col")
        nc.vector.tensor_scalar_mul(b2_col[:], b_col[:], -2.0)
        eye_f = const.tile([P, P], f32, name="eyef", tag="eyef")
        nc.sync.dma_start(eye_f[:], eye[:])
        eye_b = const.tile([P, P], bf16, name="eyeb", tag="eyeb")
        nc.scalar.copy(eye_b[:], eye_f[:])

        # ---------- augmented operand buffers ----------
        # augL rows: [0:64]=-2*xeT, [64:66]=1, [66:68]=sq hi/lo   (stationary side)
        # augR rows: [0:64]=xeT, [64:66]=(sq+1) hi/lo, [66:68]=1  (moving side)
        augL = big.tile([68, N], bf16, name="augL", tag="augL")
        augR = big.tile([68, N], bf16, name="augR", tag="augR")
        augLl = big.tile([68, B], bf16, name="augLl", tag="augLl")
        augRl = big.tile([68, B], bf16, name="augRl", tag="augRl")
        # engine ops need partition starts in {0,32,64,96}; DMA is exempt,
        # so stage the ones rows in a [2, N] tile and DMA them into place.
        onesrow = work.tile([2, B], bf16, name="onesrow", tag="onesrow")
        nc.vector.memset(onesrow[:], 1.0)
        for c8 in range(C):
            nc.sync.dma_start(augL[64:66, c8 * B:(c8 + 1) * B], onesrow[:])
            nc.sync.dma_start(augR[66:68, c8 * B:(c8 + 1) * B], onesrow[:])
        nc.sync.dma_start(augLl[64:66, :], onesrow[:])
        nc.sync.dma_start(augRl[66:68, :], onesrow[:])

        # xeT supers: psum[e, i] = sum_k W[k, e] * x[i, k]  (+ b via ACT bias)
        # x chunks are streamed from DRAM (each byte used exactly once).
        def emit_xeT(dst_R, dst_L, xsrc, nsup):
            for s in range(nsup):
                ps = psum.tile([E, SUP], f32, name="p64", tag="p64", bufs=1)
                for t in range(2):
                    xc = work.tile([P, SUP], f32, name="xc", tag="xc", bufs=3)
                    nc.sync.dma_start(
                        xc[:], xsrc[t * P:(t + 1) * P, s * SUP:(s + 1) * SUP])
                    nc.tensor.matmul(
                        ps[:], lhsT=Wsb[:, t, :], rhs=xc[:],
                        start=(t == 0), stop=(t == 1),
                    )
                sl = slice(s * SUP, (s + 1) * SUP)
                nc.scalar.activation(dst_R[0:64, sl], ps[:], AF.Identity,
                                     bias=b_col[:], scale=1.0)
                nc.scalar.activation(dst_L[0:64, sl], ps[:], AF.Identity,
                                     bias=b2_col[:], scale=-2.0)

        emit_xeT(augR, augL, xT, NSUP)
        emit_xeT(augRl, augLl, xTl, BSUP)

        # ---------- row-major bf16 xe + sq (from the SAME bf16 values) ----------
        xe_bf = big.tile([P, NT * E], bf16, name="xebf", tag="xebf")
        sq_mat = const.tile([P, NT], f32, name="sqmat", tag="sqmat")
        for it in range(NT):
            pt = psum.tile([P, E], bf16, name="pT", tag="pg", bufs=2)
            nc.tensor.transpose(pt[:], augR[0:64, it * P:(it + 1) * P],
                                eye_b[0:64, 0:64])
            nc.vector.tensor_copy(out=xe_bf[:, it * E:(it + 1) * E], in_=pt[:])
            junkE = work.tile([P, E], bf16, name="junkE", tag="junkE", bufs=2)
            nc.scalar.activation(junkE[:], pt[:], AF.Square,
                                 accum_out=sq_mat[:, it:it + 1])
        sql_mat = const.tile([P, BT], f32, name="sqlmat", tag="sqlmat")
        for it in range(BT):
            pt = psum.tile([P, E], bf16, name="pT", tag="pg", bufs=2)
            nc.tensor.transpose(pt[:], augRl[0:64, it * P:(it + 1) * P],
                                eye_b[0:64, 0:64])
            junkE = work.tile([P, E], bf16, name="junkE", tag="junkE", bufs=2)
            nc.scalar.activation(junkE[:], pt[:], AF.Square,
                                 accum_out=sql_mat[:, it:it + 1])

        # ---------- sq rows (hi/lo bf16) -> aug rows ----------
        def sq_rows(sq_tile, nt, dst_L, dst_R, nelem):
            # sq_tile [128, nt] -> T [nt, 128] -> hi/lo splits -> DMA into rows
            pt = psum.tile([nt, P], f32, name="pT2", tag="pT2", bufs=1)
            nc.tensor.transpose(pt[:], sq_tile[:], eye_f[:])
            T = work.tile([nt, P], f32, name="Tf", tag="Tf", bufs=2)
            nc.scalar.copy(T[:], pt[:])

            def hilo(src, dst0, dst1):
                hi = work.tile([nt, P], bf16, name="hi", tag="hi", bufs=2)
                nc.scalar.copy(hi[:], src[:])
                hif = work.tile([nt, P], f32, name="hif", tag="hif", bufs=2)
                nc.vector.tensor_copy(out=hif[:], in_=hi[:])
                lo = work.tile([nt, P], f32, name="lo", tag="lo", bufs=2)
                nc.vector.tensor_tensor(lo[:], src[:], hif[:], ALU.subtract)
                lob = work.tile([nt, P], bf16, name="lob", tag="lob", bufs=2)
                nc.scalar.copy(lob[:], lo[:])
                nc.sync.dma_start(dst0, hi[:])
                nc.sync.dma_start(dst1, lob[:])

            # stationary side: sq
            hilo(T, dst_L[66:67, 0:nelem], dst_L[67:68, 0:nelem])
            # moving side: sq + 1
            Tn = work.tile([nt, P], f32, name="Tn", tag="Tn", bufs=2)
            nc.vector.tensor_scalar_add(Tn[:], T[:], 1.0)
            hilo(Tn, dst_R[64:65, 0:nelem], dst_R[65:66, 0:nelem])

        sq_rows(sq_mat, NT, augL, augR, N)
        sq_rows(sql_mat, BT, augLl, augRl, B)

        # ---------- single fused pass over A ----------
        # For each jt pair: two G-matmuls produce psum [128 j, 1024] = 1+D for
        # (jt, jt+1) x the local i-super; one DVE reciprocal batches both; one
        # cast (ACT/GpSimd alternating) makes the bf16 stash tile; per-half
        # ones-matmuls accumulate column sums s on the PE.
        # isup0 tiles first so s0 can AllGather while isup1 computes.
        ones_b = const.tile([P, 1], bf16, name="onesb", tag="onesb")
        nc.vector.memset(ones_b[:], 1.0)

        stash = []  # atb[isup][pair] = [128, 1024] bf16 (jt, jt+1 halves)
        ps_s = []
        for isup in range(BSUP):
            ps_srow = psum.tile([1, SUP], f32, name=f"pss{isup}",
                                tag=f"pss{isup}", bufs=1)
            ps_s.append(ps_srow)
        for isup in range(BSUP):
            tiles = []
            for pair in range(NT // 2):
                pg = psum.tile([P, 2 * SUP], f32, name="pg", tag="pg", bufs=2)
                for h in range(2):
                    jt = 2 * pair + h
                    nc.tensor.matmul(pg[:, h * SUP:(h + 1) * SUP],
                                     lhsT=augL[:, jt * P:(jt + 1) * P],
                                     rhs=augRl[:, isup * SUP:(isup + 1) * SUP],
                                     start=True, stop=True)
                ar = work.tile([P, 2 * SUP], f32, name="ar", tag="ar", bufs=2)
                nc.vector.reciprocal_approx_fast(out=ar[:], in_=pg[:])
                atb = big.tile([P, 2 * SUP], bf16, name=f"atb{isup}_{pair}",
                               tag=f"atb{isup}_{pair}")
                if pair % 2 == 0:
                    nc.scalar.copy(atb[:], ar[:])
                else:
                    nc.gpsimd.tensor_copy(out=atb[:], in_=ar[:])
                for h in range(2):
                    jt = 2 * pair + h
                    last = (pair == NT // 2 - 1 and h == 1)
                    nc.tensor.matmul(ps_s[isup][:],
                                     lhsT=ones_b[:],
                                     rhs=atb[:, h * SUP:(h + 1) * SUP],
                                     start=(pair == 0 and h == 0),
                                     stop=last)
                tiles.append(atb)
            stash.append(tiles)

        # ---------- s -> 1/s -> split AllGather ----------
        ag_outs = []
        for isup in range(BSUP):
            srow = work.tile([1, SUP], f32, name=f"srow{isup}",
                             tag=f"srow{isup}", bufs=1)
            nc.scalar.copy(srow[:], ps_s[isup][:])
            rsrow = work.tile([1, SUP], f32, name=f"rsrow{isup}",
                              tag=f"rsrow{isup}", bufs=1)
            nc.vector.reciprocal(rsrow[:], srow[:])
            ag_in = dram.tile([SUP], f32, name=f"agin{isup}", tag=f"agin{isup}")
            ag_out = dram.tile([C * SUP], f32, name=f"agout{isup}",
                               tag=f"agout{isup}", addr_space="Shared")
            nc.sync.dma_start(ag_in[:], rsrow[:])
            nc.gpsimd.collective_compute(
                "AllGather", ALU.bypass,
                replica_groups=[list(range(C))],
                ins=[ag_in[:]], outs=[ag_out[:]],
            )
            ag_outs.append(ag_out)

        # rs arrives as [8][2][512] chunks: core, half, elem. Global j of
        # chunk (c, h) covers [c*1024 + h*512, +512) -> jt = 8c + 4h + t.
        # Load each [512] chunk as [4, 128] sbuf rows -> transpose -> rs cols.
        rs_col = const.tile([P, NT], f32, name="rscol", tag="rscol")
        ye_bf = xe_bf  # scaled in place: each jt slice written exactly once

        def emit_ye(h):
            # gather half h of every core's block, then ye for those jt
            rs_h = work.tile([4 * C, P], f32, name=f"rsh{h}", tag=f"rsh{h}",
                             bufs=1)
            for c in range(C):
                nc.sync.dma_start(rs_h[4 * c:4 * (c + 1), :],
                                  ag_outs[h][c * SUP:(c + 1) * SUP])
            prc = psum.tile([P, 4 * C], f32, name="prc", tag="pT2", bufs=1)
            nc.tensor.transpose(prc[:], rs_h[:], eye_f[0:4 * C, 0:4 * C])
            nc.scalar.copy(rs_col[:, h * 32:(h + 1) * 32], prc[:])
            for c in range(C):
                for t in range(4):
                    jt = 8 * c + 4 * h + t
                    sl = slice(jt * E, (jt + 1) * E)
                    nc.vector.tensor_scalar_mul(
                        xe_bf[:, sl], xe_bf[:, sl],
                        rs_col[:, h * 32 + 4 * c + t:h * 32 + 4 * c + t + 1])

        # rs_col layout note: column (h*32 + 4c + t) holds jt = 8c + 4h + t.
        emit_ye(0)
        emit_ye(1)

        # ---------- out^T[e, i] = sum_j ye[j, e] * A[j, i] ----------
        # jt with h=0 (jt%8<4) only need AG0, so they are emitted first and
        # overlap AG1.
        jt_order = [jt for jt in range(NT) if jt % 8 < 4] +                    [jt for jt in range(NT) if jt % 8 >= 4]
        for isup in range(BSUP):
            po = psum.tile([E, SUP], f32, name="po", tag="p64", bufs=1)
            for k, jt in enumerate(jt_order):
                atb = stash[isup][jt // 2]
                rhs = atb[:, (jt % 2) * SUP:(jt % 2 + 1) * SUP]
                nc.tensor.matmul(po[:],
                                 lhsT=ye_bf[:, jt * E:(jt + 1) * E],
                                 rhs=rhs,
                                 start=(k == 0), stop=(k == NT - 1))
            osb = work.tile([E, SUP], f32, name="osb", tag="osb", bufs=1)
            nc.scalar.copy(osb[:], po[:])
            nc.sync.dma_start(outT[:, isup * SUP:(isup + 1) * SUP], osb[:])


def _build_nc():
    _import_concourse()
    import concourse.bacc as bacc
    import concourse.tile as tile
    from concourse import mybir

    f32 = mybir.dt.float32
    nc = bacc.Bacc("TRN2", target_bir_lowering=False, debug=False,
                   num_devices=C)
    xT = nc.dram_tensor("xT", [DIN, N], f32, kind="ExternalInput").ap()
    xTl = nc.dram_tensor("xTl", [DIN, B], f32, kind="ExternalInput").ap()
    W = nc.dram_tensor("W", [DIN, E], f32, kind="ExternalInput").ap()
    b = nc.dram_tensor("b", [E, 1], f32, kind="ExternalInput").ap()
    eye = nc.dram_tensor("eye", [P, P], f32, kind="ExternalInput").ap()
    outT = nc.dram_tensor("outT", [E, B], f32, kind="ExternalOutput").ap()

    with tile.TileContext(nc) as tc:
        build_body(tc, outT, xT, xTl, W, b, eye)
    nc.compile()
    return nc


def make_in_maps(x, W_emb, b_emb):
    xT = np.ascontiguousarray(x.T).astype(np.float32)
    eye = np.eye(P, dtype=np.float32)
    bb = np.asarray(b_emb, dtype=np.float32).reshape(E, 1)
    Wf = np.asarray(W_emb, dtype=np.float32)
    in_maps = []
    for c in range(C):
        in_maps.append({
            "xT": xT,
            "xTl": np.ascontiguousarray(xT[:, c * B:(c + 1) * B]),
            "W": Wf,
            "b": bb,
            "eye": eye,
        })
    return in_maps


def kernel(x, W_emb, b_emb, _trace=False, _tmpdir=None):
    _import_concourse()
    from concourse import bass_utils

    key = "nc"
    if key not in _NC_CACHE:
        _NC_CACHE[key] = _build_nc()
    nc = _NC_CACHE[key]

    in_maps = make_in_maps(np.asarray(x), np.asarray(W_emb), np.asarray(b_emb))
    res = bass_utils.run_bass_kernel_spmd(
        nc, in_maps, core_ids=list(range(C)),
        trace=_trace, tmpdir=_tmpdir,
    )
    blocks = [np.asarray(res.results[c]["outT"]) for c in range(C)]
    outT = np.concatenate(blocks, axis=1)          # [E, N]
    out = np.ascontiguousarray(outT.T).astype(np.float32)  # [N, E]
    if _trace:
        return out, res
    return out



# revision 10
# speedup vs baseline: 1.6937x; 1.6937x over previous
"""Trainium2 Bass kernel for ContinuousDGM message passing (v2).

  xe = x @ W_emb + b_emb            [N, E]
  D  = sq_cdist(xe)                 [N, N]
  A  = 1 / (1 + D)
  W  = A / A.sum(axis=1)            (broadcast over last axis -> col-normalize)
  out = W @ xe                      [N, E]

v2 strategy (8 cores, column-block sharding of A, fully fused):
  * Core c holds A[:, c*B:(c+1)*B] as 64 bf16 tiles [128 j, 1024 i],
    produced by ONE augmented matmul per tile (psum = 1 + D), a DVE
    reciprocal_approx_fast, and ONE fused cast+row-sum:
    ACT Copy with accum_out (or GPSIMD tensor_scalar for a few tiles)
    writes the bf16 tile AND the per-j partial row sums sum_i A[j,i].
  * Row-sum partials are AllReduce'd across cores in 8 chunks of 8 jt
    tiles, pipelined with the main loop; ye = xe * (1/s) and the
    out^T accumulation matmuls trail the A-generation wavefront by 2
    chunks, so the collectives and the second pass fully overlap.
  * No separate s-matmul pass on the PE; xeT matmuls run in fp32r
    (1 cycle/row); augL is exactly -2*augR (bf16) so the diagonal of D
    cancels to sq-hi/lo precision.
"""

import os
import sys

import numpy as np

N, DIN, E = 8192, 256, 64
P = 128
C = 8
B = N // C            # 1024 columns of A per core
SUP = 512
NSUP = N // SUP       # 16
NT = N // P           # 64 jt tiles
BT = B // P           # 8
PACK = 8              # transposes packed per psum tile
NPACK = NT // PACK    # 8
CH = 8                # jt tiles per AllReduce chunk
NCH = NT // CH        # 8 chunks
SLACK = 2             # chunks the out-pass trails the A wavefront
GPS_EVERY = 8         # every GPS_EVERY-th tile cast on gpsimd (offset 3)

USE_F32R = True

_NC_CACHE = {}


def _import_concourse():
    try:
        import concourse.bacc  # noqa: F401
    except ImportError:
        for p in ("/opt/trn_rl_repo", "/root/.axon_site/_ro/trn_rl_repo"):
            if os.path.isdir(p) and p not in sys.path:
                sys.path.insert(0, p)
        import concourse.bacc  # noqa: F401


def build_body(tc, outT, xT, xTl, W, b, eye):
    from contextlib import ExitStack

    import concourse.bass as bass  # noqa: F401
    from concourse import mybir

    nc = tc.nc
    f32 = mybir.dt.float32
    f32r = mybir.dt.float32r if USE_F32R else mybir.dt.float32
    bf16 = mybir.dt.bfloat16
    AF = mybir.ActivationFunctionType
    ALU = mybir.AluOpType
    AX = mybir.AxisListType

    with ExitStack() as ctx:
        big = ctx.enter_context(tc.tile_pool(name="big", bufs=1))
        const = ctx.enter_context(tc.tile_pool(name="const", bufs=1))
        work = ctx.enter_context(tc.tile_pool(name="work", bufs=1))
        psum = ctx.enter_context(tc.tile_pool(name="psum", bufs=1, space="PSUM"))
        dram = ctx.enter_context(tc.tile_pool(name="dram", bufs=1, space="DRAM"))

        # ---------- constants ----------
        Wsb = const.tile([P, 2, E], f32r, name="Wsb", tag="Wsb")
        for t in range(2):
            nc.sync.dma_start(Wsb[:, t, :], W[t * P:(t + 1) * P, :])
        b_col = const.tile([E, 1], f32, name="bcol", tag="bcol")
        nc.sync.dma_start(b_col[:], b[:])
        eye_f = const.tile([P, P], f32, name="eyef", tag="eyef")
        nc.sync.dma_start(eye_f[:], eye[:])
        eye_b = const.tile([P, P], bf16, name="eyeb", tag="eyeb")
        nc.scalar.copy(eye_b[:], eye_f[:])

        # ---------- augmented operand buffers ----------
        # augL rows: [0:64]=-2*xeT, [64:66]=1, [66:68]=sq hi/lo   (stationary)
        # augR rows: [0:64]=xeT, [64:66]=(sq+1) hi/lo, [66:68]=1  (moving)
        augL = big.tile([68, N], bf16, name="augL", tag="augL")
        augR = big.tile([68, N], bf16, name="augR", tag="augR")
        augLl = big.tile([68, B], bf16, name="augLl", tag="augLl")
        augRl = big.tile([68, B], bf16, name="augRl", tag="augRl")
        onesrow = work.tile([2, B], bf16, name="onesrow", tag="onesrow")
        nc.vector.memset(onesrow[:], 1.0)
        for c8 in range(C):
            nc.sync.dma_start(augL[64:66, c8 * B:(c8 + 1) * B], onesrow[:])
            nc.sync.dma_start(augR[66:68, c8 * B:(c8 + 1) * B], onesrow[:])
        nc.sync.dma_start(augLl[64:66, :], onesrow[:])
        nc.sync.dma_start(augRl[66:68, :], onesrow[:])

        # ---------- xeT supers: augR[0:64] = xeT (bf16), augL = -2*augR ----
        def emit_xeT(dst_R, dst_L, xsrc, nsup):
            for s in range(nsup):
                ps = psum.tile([E, SUP], f32, name="pxe", tag="pxe", bufs=2)
                for t in range(2):
                    xc = work.tile([P, SUP], f32r, name="xc", tag="xc", bufs=2)
                    nc.sync.dma_start(
                        xc[:], xsrc[t * P:(t + 1) * P, s * SUP:(s + 1) * SUP])
                    nc.tensor.matmul(
                        ps[:], lhsT=Wsb[:, t, :], rhs=xc[:],
                        start=(t == 0), stop=(t == 1),
                    )
                sl = slice(s * SUP, (s + 1) * SUP)
                nc.scalar.activation(dst_R[0:E, sl], ps[:], AF.Identity,
                                     bias=b_col[:], scale=1.0)
                # exact -2x of the SAME bf16 values the PE multiplies
                nc.vector.tensor_scalar_mul(dst_L[0:E, sl], dst_R[0:E, sl],
                                            -2.0)

        emit_xeT(augR, augL, xT, NSUP)
        emit_xeT(augRl, augLl, xTl, 2)

        # ---------- row-major bf16 xe + sq via packed transposes ----------
        xe_bf = big.tile([P, NT * E], bf16, name="xebf", tag="xebf")
        sq_mat = const.tile([P, NT], f32, name="sqmat", tag="sqmat")
        sql_mat = const.tile([P, BT], f32, name="sqlmat", tag="sqlmat")

        xel_sb = big.tile([P, PACK * E], bf16, name="xelsb", tag="xelsb")

        def emit_pack(g, src, sqdst, want_xe):
            ptp = psum.tile([P, PACK, E], bf16, name="ptp", tag="pt", bufs=2)
            for t in range(PACK):
                jt = g * PACK + t
                nc.tensor.transpose(ptp[:, t, :], src[0:E, jt * P:(jt + 1) * P],
                                    eye_b[0:E, 0:E])
            if want_xe:
                sb = xe_bf[:, g * PACK * E:(g + 1) * PACK * E]
            else:
                sb = xel_sb[:]
            nc.scalar.activation(sb, ptp[:, :, :], AF.Copy)
            xsq = work.tile([P, PACK, E], f32, name="xsq", tag="xsq", bufs=2)
            nc.scalar.activation(xsq[:], ptp[:, :, :], AF.Square)
            nc.vector.tensor_reduce(
                out=sqdst[:, g * PACK:(g + 1) * PACK], in_=xsq[:],
                axis=AX.X, op=ALU.add)

        for g in range(NPACK):
            emit_pack(g, augR, sq_mat, True)
        emit_pack(0, augRl, sql_mat, False)

        # ---------- sq rows (hi/lo bf16) -> aug rows ----------
        def sq_rows(sq_tile, nt, dst_L, dst_R, nelem):
            pt2 = psum.tile([nt, P], f32, name="pt2", tag="pt", bufs=2)
            nc.tensor.transpose(pt2[:], sq_tile[:], eye_f[:])
            T = work.tile([nt, P], f32, name="Tf", tag="Tf", bufs=2)
            nc.scalar.copy(T[:], pt2[:])

            def hilo(src, dst0, dst1):
                hi = work.tile([nt, P], bf16, name="hi", tag="hi", bufs=2)
                nc.scalar.copy(hi[:], src[:])
                hif = work.tile([nt, P], f32, name="hif", tag="hif", bufs=2)
                nc.vector.tensor_copy(out=hif[:], in_=hi[:])
                lo = work.tile([nt, P], f32, name="lo", tag="lo", bufs=2)
                nc.vector.tensor_tensor(lo[:], src[:], hif[:], ALU.subtract)
                lob = work.tile([nt, P], bf16, name="lob", tag="lob", bufs=2)
                nc.scalar.copy(lob[:], lo[:])
                nc.sync.dma_start(dst0, hi[:])
                nc.sync.dma_start(dst1, lob[:])

            hilo(T, dst_L[66:67, 0:nelem], dst_L[67:68, 0:nelem])
            Tn = work.tile([nt, P], f32, name="Tn", tag="Tn", bufs=2)
            nc.vector.tensor_scalar_add(Tn[:], T[:], 1.0)
            hilo(Tn, dst_R[64:65, 0:nelem], dst_R[65:66, 0:nelem])

        sq_rows(sq_mat, NT, augL, augR, N)
        sq_rows(sql_mat, BT, augLl, augRl, B)

        # ---------- main fused loop ----------
        # per jt: G-matmul (1+D) -> DVE reciprocal -> fused bf16 cast +
        # row-sum partials (ACT accum_out; every GPS_EVERY-th on gpsimd).
        sprt = const.tile([P, NT], f32, name="sprt", tag="sprt")
        rs_col = const.tile([P, NT], f32, name="rscol", tag="rscol")
        stash = []
        for jt in range(NT):
            stash.append(big.tile([P, 2 * SUP], bf16, name=f"atb{jt}",
                                  tag=f"atb{jt}"))
        po0 = psum.tile([E, SUP], f32, name="po0", tag="pxe", bufs=2)
        po1 = psum.tile([E, SUP], f32, name="po1", tag="pxe", bufs=2)

        def emit_chunk_collective(k):
            agin = dram.tile([P * CH], f32, name=f"agin{k}", tag=f"agin{k}")
            agout = dram.tile([P * CH], f32, name=f"agout{k}", tag=f"agout{k}",
                              addr_space="Shared")
            nc.sync.dma_start(agin[:], sprt[:, k * CH:(k + 1) * CH])
            nc.gpsimd.collective_compute(
                "AllReduce", ALU.add,
                replica_groups=[list(range(C))],
                ins=[agin[:]], outs=[agout[:]],
            )
            s_chunk = work.tile([P, CH], f32, name=f"schunk{k}",
                                tag="schunk", bufs=2)
            nc.sync.dma_start(s_chunk[:], agout[:])
            nc.vector.reciprocal_approx_fast(
                out=rs_col[:, k * CH:(k + 1) * CH], in_=s_chunk[:])

        def emit_out_chunk(k):
            for jt in range(k * CH, (k + 1) * CH):
                sl = slice(jt * E, (jt + 1) * E)
                nc.scalar.activation(xe_bf[:, sl], xe_bf[:, sl], AF.Identity,
                                     scale=rs_col[:, jt:jt + 1])
                nc.tensor.matmul(po0[:], lhsT=xe_bf[:, sl],
                                 rhs=stash[jt][:, 0:SUP],
                                 start=(jt == 0), stop=(jt == NT - 1))
                nc.tensor.matmul(po1[:], lhsT=xe_bf[:, sl],
                                 rhs=stash[jt][:, SUP:2 * SUP],
                                 start=(jt == 0), stop=(jt == NT - 1))

        for jt in range(NT):
            pg = psum.tile([P, 2 * SUP], f32, name="pg", tag="pg", bufs=2)
            for h in range(2):
                nc.tensor.matmul(pg[:, h * SUP:(h + 1) * SUP],
                                 lhsT=augL[:, jt * P:(jt + 1) * P],
                                 rhs=augRl[:, h * SUP:(h + 1) * SUP],
                                 start=True, stop=True)
            ar = work.tile([P, 2 * SUP], f32, name="ar", tag="ar", bufs=2)
            nc.vector.reciprocal_approx_fast(out=ar[:], in_=pg[:])
            nc.scalar.activation(stash[jt][:], ar[:], AF.Copy,
                                 accum_out=sprt[:, jt:jt + 1])
            if jt % CH == CH - 1:
                k = jt // CH
                emit_chunk_collective(k)
                if k - SLACK >= 0:
                    emit_out_chunk(k - SLACK)

        for k in range(NCH - SLACK, NCH):
            emit_out_chunk(k)

        # ---------- evict out^T ----------
        for isup, po in enumerate((po0, po1)):
            osb = work.tile([E, SUP], f32, name=f"osb{isup}", tag="osb",
                            bufs=2)
            nc.scalar.copy(osb[:], po[:])
            nc.sync.dma_start(outT[:, isup * SUP:(isup + 1) * SUP], osb[:])


def _build_nc():
    _import_concourse()
    import concourse.bacc as bacc
    import concourse.tile as tile
    from concourse import mybir

    f32 = mybir.dt.float32
    f32r = mybir.dt.float32r if USE_F32R else mybir.dt.float32
    nc = bacc.Bacc("TRN2", target_bir_lowering=False, debug=False,
                   num_devices=C)
    xT = nc.dram_tensor("xT", [DIN, N], f32r, kind="ExternalInput").ap()
    xTl = nc.dram_tensor("xTl", [DIN, B], f32r, kind="ExternalInput").ap()
    W = nc.dram_tensor("W", [DIN, E], f32r, kind="ExternalInput").ap()
    b = nc.dram_tensor("b", [E, 1], f32, kind="ExternalInput").ap()
    eye = nc.dram_tensor("eye", [P, P], f32, kind="ExternalInput").ap()
    outT = nc.dram_tensor("outT", [E, B], f32, kind="ExternalOutput").ap()

    with tile.TileContext(nc) as tc:
        build_body(tc, outT, xT, xTl, W, b, eye)
    nc.compile()
    return nc


def make_in_maps(x, W_emb, b_emb):
    xT = np.ascontiguousarray(x.T).astype(np.float32)
    eye = np.eye(P, dtype=np.float32)
    bb = np.asarray(b_emb, dtype=np.float32).reshape(E, 1)
    Wf = np.asarray(W_emb, dtype=np.float32)
    in_maps = []
    for c in range(C):
        in_maps.append({
            "xT": xT,
            "xTl": np.ascontiguousarray(xT[:, c * B:(c + 1) * B]),
            "W": Wf,
            "b": bb,
            "eye": eye,
        })
    return in_maps


def kernel(x, W_emb, b_emb, _trace=False, _tmpdir=None):
    _import_concourse()
    from concourse import bass_utils

    key = "nc"
    if key not in _NC_CACHE:
        _NC_CACHE[key] = _build_nc()
    nc = _NC_CACHE[key]

    in_maps = make_in_maps(np.asarray(x), np.asarray(W_emb), np.asarray(b_emb))
    res = bass_utils.run_bass_kernel_spmd(
        nc, in_maps, core_ids=list(range(C)),
        trace=_trace, tmpdir=_tmpdir,
    )
    blocks = [np.asarray(res.results[c]["outT"]) for c in range(C)]
    outT = np.concatenate(blocks, axis=1)          # [E, N]
    out = np.ascontiguousarray(outT.T).astype(np.float32)  # [N, E]
    if _trace:
        return out, res
    return out


# revision 11
# speedup vs baseline: 1.8562x; 1.0959x over previous
"""Trainium2 Bass kernel for ContinuousDGM message passing (v3).

  xe = x @ W_emb + b_emb            [N, E]
  D  = sq_cdist(xe)                 [N, N]
  A  = 1 / (1 + D)
  W  = A / A.sum(axis=1)            (broadcast over last axis -> col-normalize)
  out = W @ xe                      [N, E]

v3 strategy (8 cores, column-block sharding of A, fully fused):
  * Core c holds A[:, c*B:(c+1)*B] as 64 bf16 tiles [128 j, 1024 i]:
    one augmented G-matmul produces psum = 1 + D, DVE
    reciprocal_approx_fast runs IN PLACE on the psum tile, and one ACT
    Copy with accum_out writes the bf16 stash tile AND the per-j row-sum
    partials sum_i A[j,i].
  * Setup is emitted pack-by-pack (2 supers -> 8 transposes -> sq ->
    sq-row DMA) so the main loop's G-matmuls unlock progressively and
    overlap the x load.
  * Row-sum partials AllReduce across cores in chunks ([8]*7 + [4,4] jt
    tiles), pipelined with the main loop; ye = xe * (1/s) and the out^T
    matmuls trail the A wavefront by 2 chunks.
"""

import os
import sys

import numpy as np

N, DIN, E = 8192, 256, 64
P = 128
C = 8
B = N // C            # 1024 columns of A per core
SUP = 512
NSUP = N // SUP       # 16
NT = N // P           # 64 jt tiles
BT = B // P           # 8
PACK = 8              # transposes / jt tiles per pack
NPACK = NT // PACK    # 8
CHUNKS = [8] * 7 + [4, 4]   # jt tiles per AllReduce chunk
SLACK = 2             # chunks the out-pass trails the A wavefront

USE_F32R = True

_NC_CACHE = {}


def _import_concourse():
    try:
        import concourse.bacc  # noqa: F401
    except ImportError:
        for p in ("/opt/trn_rl_repo", "/root/.axon_site/_ro/trn_rl_repo"):
            if os.path.isdir(p) and p not in sys.path:
                sys.path.insert(0, p)
        import concourse.bacc  # noqa: F401


def build_body(tc, outT, xT, xTl, W, b, eye):
    from contextlib import ExitStack

    import concourse.bass as bass  # noqa: F401
    from concourse import mybir

    nc = tc.nc
    f32 = mybir.dt.float32
    f32r = mybir.dt.float32r if USE_F32R else mybir.dt.float32
    bf16 = mybir.dt.bfloat16
    AF = mybir.ActivationFunctionType
    ALU = mybir.AluOpType
    AX = mybir.AxisListType

    with ExitStack() as ctx:
        big = ctx.enter_context(tc.tile_pool(name="big", bufs=1))
        const = ctx.enter_context(tc.tile_pool(name="const", bufs=1))
        work = ctx.enter_context(tc.tile_pool(name="work", bufs=1))
        psum = ctx.enter_context(tc.tile_pool(name="psum", bufs=1, space="PSUM"))
        dram = ctx.enter_context(tc.tile_pool(name="dram", bufs=1, space="DRAM"))

        # ---------- constants ----------
        Wsb = const.tile([P, 2, E], f32r, name="Wsb", tag="Wsb")
        for t in range(2):
            nc.sync.dma_start(Wsb[:, t, :], W[t * P:(t + 1) * P, :])
        b_col = const.tile([E, 1], f32, name="bcol", tag="bcol")
        nc.sync.dma_start(b_col[:], b[:])
        eye_f = const.tile([P, P], f32, name="eyef", tag="eyef")
        nc.sync.dma_start(eye_f[:], eye[:])
        eye_b = const.tile([P, P], bf16, name="eyeb", tag="eyeb")
        nc.scalar.copy(eye_b[:], eye_f[:])

        # ---------- augmented operand buffers ----------
        # augL rows: [0:64]=-2*xeT, [64:66]=1, [66:68]=sq hi/lo   (stationary)
        # augR rows: [0:64]=xeT, [64:66]=(sq+1) hi/lo, [66:68]=1  (moving)
        augL = big.tile([68, N], bf16, name="augL", tag="augL")
        augR = big.tile([68, N], bf16, name="augR", tag="augR")
        augLl = big.tile([68, B], bf16, name="augLl", tag="augLl")
        augRl = big.tile([68, B], bf16, name="augRl", tag="augRl")
        onesrow = work.tile([2, B], bf16, name="onesrow", tag="onesrow")
        nc.vector.memset(onesrow[:], 1.0)
        for c8 in range(C):
            nc.sync.dma_start(augL[64:66, c8 * B:(c8 + 1) * B], onesrow[:])
            nc.sync.dma_start(augR[66:68, c8 * B:(c8 + 1) * B], onesrow[:])
        nc.sync.dma_start(augLl[64:66, :], onesrow[:])
        nc.sync.dma_start(augRl[66:68, :], onesrow[:])

        xe_bf = big.tile([P, NT * E], bf16, name="xebf", tag="xebf")
        xel_sb = big.tile([P, PACK * E], bf16, name="xelsb", tag="xelsb")
        sq_mat = const.tile([P, NT], f32, name="sqmat", tag="sqmat")
        sql_mat = const.tile([P, BT], f32, name="sqlmat", tag="sqlmat")

        # ---------- building blocks ----------
        def emit_super(dst_R, dst_L, xsrc, s):
            ps = psum.tile([E, SUP], f32, name="pxe", tag="aux", bufs=2)
            for t in range(2):
                xc = work.tile([P, SUP], f32r, name="xc", tag="xc", bufs=4)
                nc.sync.dma_start(
                    xc[:], xsrc[t * P:(t + 1) * P, s * SUP:(s + 1) * SUP])
                nc.tensor.matmul(
                    ps[:], lhsT=Wsb[:, t, :], rhs=xc[:],
                    start=(t == 0), stop=(t == 1),
                )
            sl = slice(s * SUP, (s + 1) * SUP)
            nc.scalar.activation(dst_R[0:E, sl], ps[:], AF.Identity,
                                 bias=b_col[:], scale=1.0)
            nc.vector.tensor_scalar_mul(dst_L[0:E, sl], dst_R[0:E, sl], -2.0)

        def emit_pack(g, src, sqdst, want_xe):
            # 8 transposes -> [128, 8, 64] psum; copy row-major xe; squares
            ptp = psum.tile([P, PACK, E], bf16, name="ptp", tag="aux", bufs=2)
            for t in range(PACK):
                jt = g * PACK + t
                nc.tensor.transpose(ptp[:, t, :], src[0:E, jt * P:(jt + 1) * P],
                                    eye_b[0:E, 0:E])
            if want_xe:
                sb = xe_bf[:, g * PACK * E:(g + 1) * PACK * E]
            else:
                sb = xel_sb[:]
            nc.scalar.activation(sb, ptp[:, :, :], AF.Copy)
            xsq = work.tile([P, PACK, E], f32, name="xsq", tag="xsq", bufs=2)
            nc.scalar.activation(xsq[:], ptp[:, :, :], AF.Square)
            nc.vector.tensor_reduce(
                out=sqdst[:, g * PACK:(g + 1) * PACK], in_=xsq[:],
                axis=AX.X, op=ALU.add)

        def emit_sq_rows(sq_slice, nt, dst_L, dst_R, col0):
            # sq_slice [128, nt] -> [nt, 128] -> hi/lo -> aug rows cols
            # [col0*128, (col0+nt)*128)
            pt2 = psum.tile([nt, P], f32, name="pt2", tag="aux", bufs=2)
            nc.tensor.transpose(pt2[:], sq_slice, eye_f[:])
            T = work.tile([nt, P], f32, name="Tf", tag="Tf", bufs=2)
            nc.scalar.copy(T[:], pt2[:])
            csl0 = col0 * P
            csl1 = (col0 + nt) * P

            def hilo(src, dst0, dst1):
                hi = work.tile([nt, P], bf16, name="hi", tag="hi", bufs=2)
                nc.scalar.copy(hi[:], src[:])
                hif = work.tile([nt, P], f32, name="hif", tag="hif", bufs=2)
                nc.vector.tensor_copy(out=hif[:], in_=hi[:])
                lo = work.tile([nt, P], f32, name="lo", tag="lo", bufs=2)
                nc.vector.tensor_tensor(lo[:], src[:], hif[:], ALU.subtract)
                lob = work.tile([nt, P], bf16, name="lob", tag="lob", bufs=2)
                nc.scalar.copy(lob[:], lo[:])
                nc.sync.dma_start(dst0, hi[:])
                nc.sync.dma_start(dst1, lob[:])

            hilo(T, dst_L[66:67, csl0:csl1], dst_L[67:68, csl0:csl1])
            Tn = work.tile([nt, P], f32, name="Tn", tag="Tn", bufs=2)
            nc.vector.tensor_scalar_add(Tn[:], T[:], 1.0)
            hilo(Tn, dst_R[64:65, csl0:csl1], dst_R[65:66, csl0:csl1])

        # ---------- local block first (unblocks every G-matmul's rhs) ----
        for s in range(2):
            emit_super(augRl, augLl, xTl, s)
        emit_pack(0, augRl, sql_mat, False)
        emit_sq_rows(sql_mat[:, 0:BT], BT, augLl, augRl, 0)

        # ---------- global supers, pack by pack ----------
        for g in range(NPACK):
            emit_super(augR, augL, xT, 2 * g)
            emit_super(augR, augL, xT, 2 * g + 1)
            emit_pack(g, augR, sq_mat, True)
            emit_sq_rows(sq_mat[:, g * PACK:(g + 1) * PACK], PACK,
                         augL, augR, g * PACK)

        # ---------- main fused loop ----------
        sprt = const.tile([P, NT], f32, name="sprt", tag="sprt")
        rs_col = const.tile([P, NT], f32, name="rscol", tag="rscol")
        stash = []
        for jt in range(NT):
            stash.append(big.tile([P, 2 * SUP], bf16, name=f"atb{jt}",
                                  tag=f"atb{jt}"))
        po0 = psum.tile([E, SUP], f32, name="po0", tag="aux", bufs=2)
        po1 = psum.tile([E, SUP], f32, name="po1", tag="aux", bufs=2)

        cbounds = [0]
        for w in CHUNKS:
            cbounds.append(cbounds[-1] + w)

        def emit_chunk_collective(k):
            j0, j1 = cbounds[k], cbounds[k + 1]
            w = j1 - j0
            agin = dram.tile([P * w], f32, name=f"agin{k}", tag=f"agin{k}")
            agout = dram.tile([P * w], f32, name=f"agout{k}", tag=f"agout{k}",
                              addr_space="Shared")
            nc.sync.dma_start(agin[:], sprt[:, j0:j1])
            nc.gpsimd.collective_compute(
                "AllReduce", ALU.add,
                replica_groups=[list(range(C))],
                ins=[agin[:]], outs=[agout[:]],
            )
            s_chunk = work.tile([P, w], f32, name=f"schunk{k}",
                                tag="schunk", bufs=2)
            nc.sync.dma_start(s_chunk[:], agout[:])
            nc.vector.reciprocal_approx_fast(
                out=rs_col[:, j0:j1], in_=s_chunk[:])

        def emit_out_chunk(k):
            for jt in range(cbounds[k], cbounds[k + 1]):
                sl = slice(jt * E, (jt + 1) * E)
                nc.scalar.activation(xe_bf[:, sl], xe_bf[:, sl], AF.Identity,
                                     scale=rs_col[:, jt:jt + 1])
                nc.tensor.matmul(po0[:], lhsT=xe_bf[:, sl],
                                 rhs=stash[jt][:, 0:SUP],
                                 start=(jt == 0), stop=(jt == NT - 1))
                nc.tensor.matmul(po1[:], lhsT=xe_bf[:, sl],
                                 rhs=stash[jt][:, SUP:2 * SUP],
                                 start=(jt == 0), stop=(jt == NT - 1))

        nchunks = len(CHUNKS)
        k = 0
        for jt in range(NT):
            pg = psum.tile([P, 2 * SUP], f32, name="pg", tag="pg", bufs=3)
            for h in range(2):
                nc.tensor.matmul(pg[:, h * SUP:(h + 1) * SUP],
                                 lhsT=augL[:, jt * P:(jt + 1) * P],
                                 rhs=augRl[:, h * SUP:(h + 1) * SUP],
                                 start=True, stop=True)
            # reciprocal in place on the psum tile
            nc.vector.reciprocal_approx_fast(out=pg[:], in_=pg[:])
            nc.scalar.activation(stash[jt][:], pg[:], AF.Copy,
                                 accum_out=sprt[:, jt:jt + 1])
            if k < nchunks and jt == cbounds[k + 1] - 1:
                emit_chunk_collective(k)
                if k - SLACK >= 0:
                    emit_out_chunk(k - SLACK)
                k += 1

        for kk in range(nchunks - SLACK, nchunks):
            emit_out_chunk(kk)

        # ---------- evict out^T ----------
        for isup, po in enumerate((po0, po1)):
            osb = work.tile([E, SUP], f32, name=f"osb{isup}", tag="osb",
                            bufs=2)
            nc.scalar.copy(osb[:], po[:])
            nc.sync.dma_start(outT[:, isup * SUP:(isup + 1) * SUP], osb[:])


def _build_nc():
    _import_concourse()
    import concourse.bacc as bacc
    import concourse.tile as tile
    from concourse import mybir

    f32 = mybir.dt.float32
    f32r = mybir.dt.float32r if USE_F32R else mybir.dt.float32
    nc = bacc.Bacc("TRN2", target_bir_lowering=False, debug=False,
                   num_devices=C)
    xT = nc.dram_tensor("xT", [DIN, N], f32r, kind="ExternalInput").ap()
    xTl = nc.dram_tensor("xTl", [DIN, B], f32r, kind="ExternalInput").ap()
    W = nc.dram_tensor("W", [DIN, E], f32r, kind="ExternalInput").ap()
    b = nc.dram_tensor("b", [E, 1], f32, kind="ExternalInput").ap()
    eye = nc.dram_tensor("eye", [P, P], f32, kind="ExternalInput").ap()
    outT = nc.dram_tensor("outT", [E, B], f32, kind="ExternalOutput").ap()

    with tile.TileContext(nc) as tc:
        build_body(tc, outT, xT, xTl, W, b, eye)
    nc.compile()
    return nc


def make_in_maps(x, W_emb, b_emb):
    xT = np.ascontiguousarray(x.T).astype(np.float32)
    eye = np.eye(P, dtype=np.float32)
    bb = np.asarray(b_emb, dtype=np.float32).reshape(E, 1)
    Wf = np.asarray(W_emb, dtype=np.float32)
    in_maps = []
    for c in range(C):
        in_maps.append({
            "xT": xT,
            "xTl": np.ascontiguousarray(xT[:, c * B:(c + 1) * B]),
            "W": Wf,
            "b": bb,
            "eye": eye,
        })
    return in_maps


def kernel(x, W_emb, b_emb, _trace=False, _tmpdir=None):
    _import_concourse()
    from concourse import bass_utils

    key = "nc"
    if key not in _NC_CACHE:
        _NC_CACHE[key] = _build_nc()
    nc = _NC_CACHE[key]

    in_maps = make_in_maps(np.asarray(x), np.asarray(W_emb), np.asarray(b_emb))
    res = bass_utils.run_bass_kernel_spmd(
        nc, in_maps, core_ids=list(range(C)),
        trace=_trace, tmpdir=_tmpdir,
    )
    blocks = [np.asarray(res.results[c]["outT"]) for c in range(C)]
    outT = np.concatenate(blocks, axis=1)          # [E, N]
    out = np.ascontiguousarray(outT.T).astype(np.float32)  # [N, E]
    if _trace:
        return out, res
    return out
